# revision 12
# baseline (speedup 1.0000x reference)
"""Trainium2 Bass kernel for nn_Dynamics: 3-layer LSTM (H=512, B=256, T=128)
+ final linear, data-parallel over batch across 8 NeuronCores.

Per-core design (B_loc=32):
  - All matmuls fp16 (fp32 PSUM accumulate), elementwise fp32, c-state fp32.
  - Gates computed transposed: gatesT [2048, 32] as 16 PSUM chunks [128, 32];
    weight strips [128,128] stationary (fp16 FWL), hT [128,32] moving.
  - Gate order reordered host-side to [i, f, o, g] so sigmoid covers one
    contiguous [128, 384] region and tanh one [128, 128].
  - Input contributions xg precomputed blockwise (S steps) into SBUF via
    big-N matmuls from the previous layer's h-sequence blocks; bias folded
    into the PSUM->SBUF copy on the scalar engine (per-partition bias AP).
  - 3 layers software-pipelined in a skewed wavefront: at wave w the core
    runs recurrence blocks (l0, w), (l1, w-1), (l2, w-2) interleaved per
    step, hiding each layer's activation/elementwise chain under the other
    layers' matmuls. Fully static unroll (no loop barriers).
  - Host-side numpy does all layout prep: x transpose to [features, (t,b)],
    weight transpose/reorder/chunking, bias sums, fp16 casts, padding.

Self-contained: only needs numpy + the installed concourse/jax environment.
"""

import numpy as np

STATE_DIM, ACTION_DIM, HIDDEN, NUM_LAYERS = 64, 32, 512, 3
B, T = 256, 128
NCORES = 8
BLOC = B // NCORES          # 32 batch rows per core
S = 8                       # steps per block
NBLK = T // S
H4 = 4 * HIDDEN
NK = HIDDEN // 128          # 4 k-chunks of the hidden dim
NM = H4 // 128              # 16 m-chunks of the gate dim

# reorder 4H rows from (i,f,g,o) to (i,f,o,g)
GATE_PERM = np.r_[0:1024, 1536:2048, 1024:1536]

# --- optimization flags ---
OPT_GP_ELEM = False    # prods/cadd elementwise on GpSimd instead of DVE
OPT_COPY_DVE = False   # alternate precompute PSUM->SBUF copies ACT/DVE
OPT_BF16 = False       # bf16 matmul operands instead of fp16
OPT_PRE_SPREAD = False  # interleave precompute m-chunks between steps

# ---------------------------------------------------------------------------
# Tile/walrus compatibility patches
# ---------------------------------------------------------------------------


def _apply_tile_patches():
    import concourse.mybir as mybir
    import concourse.tile as tile
    from concourse.vector_clock import ScopedClock

    n_spill = 24

    def _patched_dab(self, tick_clock, wait_clock):
        nc = self.nc
        nops = [nc.sync.nop(hint=f"drain_spill{i}") for i in range(n_spill)]
        drain_inst = nc.sync.drain()
        wait_clock.add_sem_waits(
            drain_inst.ins, ScopedClock({None: tick_clock.global_clock})
        )
        si = drain_inst.ins.sync_info
        waits = list(si.on_wait) if si and si.on_wait else []
        ups = list(si.on_update) if si and si.on_update else []
        if len(waits) > 1:
            spill, keep = waits[:-1], waits[-1:]
            assert len(spill) <= n_spill
            for nop, w in zip(nops, spill):
                nsi = nop.ins.sync_info
                nups = list(nsi.on_update) if nsi and nsi.on_update else []
                nop.ins.sync_info = mybir.SyncInfo(on_wait=[w], on_update=nups)
            drain_inst.ins.sync_info = mybir.SyncInfo(on_wait=keep, on_update=ups)
        nc.all_engine_barrier()
        popped = nc._tile_sem_poison_stack.pop()
        assert popped is self._sem_poison
        nc.clear_and_free_semaphores(list(self.sems.allocated().values()))
        nc.all_engine_barrier()

    tile.TileContext._drain_and_barrier = _patched_dab


_SPILL_N = [0]


def _spill_excess_waits(nc, limit=1):
    """This walrus build accepts only `limit` sync-waits per instruction;
    move excess onto injected same-engine NoOps placed just before."""
    import concourse.mybir as mybir

    for f in nc.m.functions:
        for bb in f.blocks:
            out = []
            changed = False
            for inst in bb.instructions:
                si = inst.sync_info
                waits = list(si.on_wait) if si and si.on_wait else []
                if len(waits) > limit and inst.engine != mybir.EngineType.Unassigned:
                    for w in waits[:-limit]:
                        _SPILL_N[0] += 1
                        nop = mybir.InstNoOp(
                            name=f"wait-spill-{_SPILL_N[0]}", ins=[], outs=[]
                        )
                        nop.engine = inst.engine
                        nop.sync_info = mybir.SyncInfo(on_wait=[w], on_update=[])
                        nc.register_instruction(nop)
                        out.append(nop)
                    inst.sync_info = mybir.SyncInfo(
                        on_wait=waits[-limit:],
                        on_update=list(si.on_update) if si.on_update else [],
                    )
                    changed = True
                out.append(inst)
            if changed:
                bb.instructions = out


# ---------------------------------------------------------------------------
# Bass program
# ---------------------------------------------------------------------------


def build_core_program(t_steps=T, s_blk=S, reps=1):
    import concourse.bass as bass
    import concourse.mybir as mybir
    import concourse.tile as tile

    _apply_tile_patches()

    F16 = mybir.dt.bfloat16 if OPT_BF16 else mybir.dt.float16
    F32 = mybir.dt.float32
    AF = mybir.ActivationFunctionType
    OP = mybir.AluOpType

    nblk = t_steps // s_blk
    ntok_blk = s_blk * BLOC

    nc = bass.Bass("TRN2", target_bir_lowering=False, debug=False,
                   num_devices=NCORES)

    xT = nc.declare_dram_parameter("xT", [128, t_steps * BLOC], F16,
                                   isOutput=False)
    whs, wis, bsums = [], [], []
    for l in range(NUM_LAYERS):
        whs.append(nc.declare_dram_parameter(
            f"wh{l}", [128, NK, H4], F16, isOutput=False))
        nki = 1 if l == 0 else NK
        wis.append(nc.declare_dram_parameter(
            f"wi{l}", [128, nki, H4], F16, isOutput=False))
        bsums.append(nc.declare_dram_parameter(
            f"bsum{l}", [128, NM], F32, isOutput=False))
    lin_w_in = nc.declare_dram_parameter("lin_wT", [128, NK, STATE_DIM], F16,
                                         isOutput=False)
    lin_b_in = nc.declare_dram_parameter("lin_b", [STATE_DIM, 1], F32,
                                         isOutput=False)
    outT = nc.declare_dram_parameter("outT", [STATE_DIM, BLOC], F32,
                                     isOutput=True)

    with tile.TileContext(nc) as tc:
        import contextlib
        with contextlib.ExitStack() as ctx:
            wp = ctx.enter_context(tc.tile_pool(name="weights", bufs=1))
            sp = ctx.enter_context(tc.tile_pool(name="state", bufs=1))
            pgates = ctx.enter_context(
                tc.tile_pool(name="pgates", bufs=4, space="PSUM"))
            ppre = ctx.enter_context(
                tc.tile_pool(name="ppre", bufs=2, space="PSUM"))
            plin = ctx.enter_context(
                tc.tile_pool(name="plin", bufs=1, space="PSUM"))
            xgp = [ctx.enter_context(tc.tile_pool(name=f"xg{l}", bufs=2))
                   for l in range(NUM_LAYERS)]
            hbp = [ctx.enter_context(tc.tile_pool(name=f"hb{l}", bufs=3))
                   for l in range(NUM_LAYERS)]
            tp = ctx.enter_context(tc.tile_pool(name="tmp", bufs=3))

            # --- load weights / constants ---
            wh_t, wi_t, bs_t = [], [], []
            for l in range(NUM_LAYERS):
                w = wp.tile([128, NK, H4], F16, tag=f"wh{l}")
                nc.gpsimd.dma_start(out=w, in_=whs[l][:, :, :])
                wh_t.append(w)
                nki = 1 if l == 0 else NK
                wi = wp.tile([128, nki, H4], F16, tag=f"wi{l}")
                nc.gpsimd.dma_start(out=wi, in_=wis[l][:, :, :])
                wi_t.append(wi)
                bs = wp.tile([128, NM], F32, tag=f"bs{l}")
                nc.sync.dma_start(out=bs, in_=bsums[l][:, :])
                bs_t.append(bs)
            xT_t = wp.tile([128, t_steps * BLOC], F16, tag="xT")
            nc.gpsimd.dma_start(out=xT_t, in_=xT[:, :])
            lw_t = wp.tile([128, NK, STATE_DIM], F16, tag="lw")
            nc.sync.dma_start(out=lw_t, in_=lin_w_in[:, :, :])
            lb_t = wp.tile([STATE_DIM, 1], F32, tag="lb")
            nc.sync.dma_start(out=lb_t, in_=lin_b_in[:, :])

            # --- persistent state ---
            Xs = []   # [tanh_g | c] fp32
            for l in range(NUM_LAYERS):
                X = sp.tile([128, 256], F32, tag=f"X{l}")
                nc.vector.memset(X, 0.0)
                Xs.append(X)

            # per-layer rolling H blocks and xg blocks (python-side ring)
            h_blocks = [[None] * nblk for _ in range(NUM_LAYERS)]
            xg_blocks = [[None] * nblk for _ in range(NUM_LAYERS)]

            def precompute_alloc(l, j):
                xg = xgp[l].tile([128, s_blk, H4 // NK], F16, tag=f"xg{l}")
                xg_blocks[l][j] = xg
                return xg

            def precompute_m(l, j, m, xg):
                """one m-chunk of xg block j for layer l."""
                nki = 1 if l == 0 else NK
                ps = ppre.tile([128, ntok_blk], F32, tag="pre")
                for k in range(nki):
                    if l == 0:
                        rhs = xT_t[:, j * ntok_blk:(j + 1) * ntok_blk]
                    else:
                        rhs = h_blocks[l - 1][j][:, k, :, :].rearrange(
                            "p t b -> p (t b)")
                    nc.tensor.matmul(
                        ps[:, :],
                        wi_t[l][:, k, 128 * m:128 * m + 128],
                        rhs,
                        start=(k == 0), stop=(k == nki - 1),
                    )
                if OPT_COPY_DVE and m % 2 == 1:
                    nc.vector.tensor_scalar_add(
                        out=xg[:, :, 32 * m:32 * m + 32],
                        in0=ps.rearrange("p (t b) -> p t b", t=s_blk),
                        scalar1=bs_t[l][:, m:m + 1],
                    )
                else:
                    nc.scalar.activation(
                        out=xg[:, :, 32 * m:32 * m + 32],
                        in_=ps.rearrange("p (t b) -> p t b", t=s_blk),
                        func=AF.Identity,
                        bias=bs_t[l][:, m:m + 1],
                    )

            def precompute(l, j):
                """xg block j for layer l -> SBUF fp16 [128, s_blk, 512]."""
                xg = precompute_alloc(l, j)
                for m in range(NM):
                    precompute_m(l, j, m, xg)

            def step(l, j, tau):
                """Recurrence step tau of block j, layer l."""
                X = Xs[l]
                first = (j == 0 and tau == 0)
                hb = h_blocks[l][j]
                if tau == 0:
                    hb = hbp[l].tile([128, NK, s_blk, BLOC], F16, tag=f"hb{l}")
                    h_blocks[l][j] = hb
                xg_step = xg_blocks[l][j][:, tau, :]
                if not first:
                    if tau == 0:
                        h_prev = h_blocks[l][j - 1][:, :, s_blk - 1, :]
                    else:
                        h_prev = h_blocks[l][j][:, :, tau - 1, :]
                    ps = pgates.tile([128, H4 // NK], F32, tag="gates")
                    for m in range(NM):
                        for k in range(NK):
                            nc.tensor.matmul(
                                ps[:, 32 * m:32 * m + 32],
                                wh_t[l][:, k, 128 * m:128 * m + 128],
                                h_prev[:, k, :],
                                start=(k == 0), stop=(k == NK - 1),
                            )
                    gsum = tp.tile([128, 512], F32, tag="gsum")
                    nc.vector.tensor_tensor(
                        out=gsum, in0=ps, in1=xg_step, op=OP.add)
                    gsrc = gsum
                else:
                    gsrc = xg_step
                sig = tp.tile([128, 384], F32, tag="sig")
                nc.scalar.activation(out=sig, in_=gsrc[:, 0:384],
                                     func=AF.Sigmoid)
                nc.scalar.activation(out=X[:, 0:128], in_=gsrc[:, 384:512],
                                     func=AF.Tanh)
                prods = tp.tile([128, 256], F32, tag="prods")
                elem = nc.gpsimd if OPT_GP_ELEM else nc.vector
                elem.tensor_tensor(out=prods, in0=X, in1=sig[:, 0:256],
                                   op=OP.mult)
                elem.tensor_tensor(out=X[:, 128:256], in0=prods[:, 0:128],
                                   in1=prods[:, 128:256], op=OP.add)
                tcn = tp.tile([128, 128], F32, tag="tcn")
                nc.scalar.activation(out=tcn, in_=X[:, 128:256], func=AF.Tanh)
                nc.vector.tensor_tensor(
                    out=hb[:, :, tau, :],
                    in0=sig[:, 256:384].rearrange("p (k b) -> p k b", k=NK),
                    in1=tcn.rearrange("p (k b) -> p k b", k=NK),
                    op=OP.mult)

            # --- skewed wavefront over blocks ---
            for _rep in range(reps):
                precompute(0, 0)
                for w in range(nblk + NUM_LAYERS - 1):
                    p0_tile = None
                    if OPT_PRE_SPREAD and w + 1 < nblk:
                        p0_tile = precompute_alloc(0, w + 1)
                    for tau in range(s_blk):
                        for l in range(NUM_LAYERS):
                            j = w - l
                            if 0 <= j < nblk:
                                step(l, j, tau)
                        if p0_tile is not None:
                            for m in range(2 * tau, 2 * tau + 2):
                                precompute_m(0, w + 1, m, p0_tile)
                    # remaining precompute at wave end (P1/P2 depend on the
                    # h blocks produced by this wave's steps)
                    if w + 1 < nblk and p0_tile is None:
                        precompute(0, w + 1)
                    for l in range(1, NUM_LAYERS):
                        j = w - l + 1
                        if 0 <= j < nblk:
                            precompute(l, j)

            # --- final linear: outT = lin_w @ h_last + lin_b ---
            h_last = h_blocks[NUM_LAYERS - 1][nblk - 1][:, :, s_blk - 1, :]
            pl = plin.tile([STATE_DIM, BLOC], F32, tag="lin")
            for k in range(NK):
                nc.tensor.matmul(
                    pl[:, :],
                    lw_t[:, k, :],
                    h_last[:, k, :],
                    start=(k == 0), stop=(k == NK - 1),
                )
            osb = tp.tile([STATE_DIM, BLOC], F32, tag="osb")
            nc.scalar.activation(out=osb, in_=pl, func=AF.Identity,
                                 bias=lb_t[:, 0:1])
            nc.sync.dma_start(out=outT[:, :], in_=osb)

    _spill_excess_waits(nc, limit=1)
    return nc


# ---------------------------------------------------------------------------
# Host-side input prep
# ---------------------------------------------------------------------------


def prep_core_inputs(inputs, core, t_steps=T):
    if OPT_BF16:
        import ml_dtypes
        f16 = ml_dtypes.bfloat16
    else:
        f16 = np.float16
    state = inputs["state"][core * BLOC:(core + 1) * BLOC, :t_steps]
    action = inputs["action"][core * BLOC:(core + 1) * BLOC, :t_steps]
    x = np.concatenate([state, action], axis=-1)  # [BLOC, t, 96]
    in_dim = x.shape[-1]
    xT = np.zeros((128, t_steps * BLOC), f16)
    xT[:in_dim] = x.transpose(2, 1, 0).reshape(in_dim, t_steps * BLOC)

    m = {"xT": xT}
    for l in range(NUM_LAYERS):
        w_ih = inputs[f"w_ih_{l}"][GATE_PERM]      # [2048, d_in]
        w_hh = inputs[f"w_hh_{l}"][GATE_PERM]      # [2048, 512]
        bsum = (inputs[f"b_ih_{l}"] + inputs[f"b_hh_{l}"])[GATE_PERM]
        wt = w_hh.T.reshape(NK, 128, H4).transpose(1, 0, 2)
        m[f"wh{l}"] = wt.astype(f16)
        if l == 0:
            wi = np.zeros((128, 1, H4), np.float32)
            wi[:w_ih.shape[1], 0] = w_ih.T
        else:
            wi = w_ih.T.reshape(NK, 128, H4).transpose(1, 0, 2)
        m[f"wi{l}"] = wi.astype(f16)
        m[f"bsum{l}"] = np.ascontiguousarray(
            bsum.reshape(NM, 128).T).astype(np.float32)
    m["lin_wT"] = np.ascontiguousarray(
        inputs["lin_w"].T.reshape(NK, 128, STATE_DIM).transpose(1, 0, 2)
    ).astype(f16)
    m["lin_b"] = inputs["lin_b"].reshape(STATE_DIM, 1).astype(np.float32)
    return m


# ---------------------------------------------------------------------------
# PJRT runner (axon path), adapted from concourse.bass2jax.run_bass_via_pjrt
# ---------------------------------------------------------------------------


class BassRunner:
    def __init__(self, nc, n_cores):
        import jax
        from jax.sharding import Mesh, PartitionSpec
        from jax.experimental.shard_map import shard_map
        import concourse.mybir as mybir
        from concourse.bass2jax import (
            _bass_exec_p, install_neuronx_cc_hook, partition_id_tensor)

        install_neuronx_cc_hook()
        self.jax = jax
        self.nc = nc
        self.n_cores = n_cores
        partition_name = (
            nc.partition_id_tensor.name if nc.partition_id_tensor else None)

        in_names, out_names, out_avals, zero_outs = [], [], [], []
        for alloc in nc.m.functions[0].allocations:
            if not isinstance(alloc, mybir.MemoryLocationSet):
                continue
            name = alloc.memorylocations[0].name
            if alloc.kind == "ExternalInput":
                if name != partition_name:
                    in_names.append(name)
            elif alloc.kind == "ExternalOutput":
                out_names.append(name)
                shape = tuple(alloc.tensor_shape)
                dtype = mybir.dt.np(alloc.dtype)
                out_avals.append(jax.core.ShapedArray(shape, dtype))
                zero_outs.append(np.zeros(shape, dtype))
        self.in_names = in_names
        self.out_names = out_names
        self.zero_outs = zero_outs
        n_params, n_outs = len(in_names), len(out_avals)

        all_in_names = list(in_names) + list(out_names)
        if partition_name is not None:
            all_in_names.append(partition_name)

        def _body(*args):
            operands = list(args)
            if partition_name is not None:
                operands.append(partition_id_tensor())
            return tuple(_bass_exec_p.bind(
                *operands,
                out_avals=tuple(out_avals),
                in_names=tuple(all_in_names),
                out_names=tuple(out_names),
                lowering_input_output_aliases=(),
                sim_require_finite=True,
                sim_require_nnan=True,
                nc=nc,
            ))

        devices = jax.devices()[:n_cores]
        mesh = Mesh(np.asarray(devices), ("core",))
        self.fn = jax.jit(
            shard_map(_body, mesh=mesh,
                      in_specs=(PartitionSpec("core"),) * (n_params + n_outs),
                      out_specs=(PartitionSpec("core"),) * n_outs,
                      check_rep=False),
            donate_argnums=tuple(range(n_params, n_params + n_outs)),
            keep_unused=True,
        )

    def put_inputs(self, in_maps):
        return [
            self.jax.device_put(np.concatenate(
                [np.asarray(m[name]) for m in in_maps], axis=0))
            for name in self.in_names
        ]

    def _zeros(self):
        jnp = self.jax.numpy
        return [jnp.zeros((self.n_cores * z.shape[0], *z.shape[1:]), z.dtype)
                for z in self.zero_outs]

    def run(self, dev_inputs):
        outs = self.fn(*dev_inputs, *self._zeros())
        self.jax.block_until_ready(outs)
        return outs

    def time_run(self, dev_inputs, iters=10, warmup=2):
        import time as _time
        for _ in range(warmup):
            self.run(dev_inputs)
        ts = []
        for _ in range(iters):
            zs = self._zeros()
            self.jax.block_until_ready(zs)
            t0 = _time.perf_counter()
            outs = self.fn(*dev_inputs, *zs)
            self.jax.block_until_ready(outs)
            ts.append(_time.perf_counter() - t0)
        return min(ts), ts


# ---------------------------------------------------------------------------
# Public entry point
# ---------------------------------------------------------------------------

_CACHE = {}


def _get_runner(t_steps=T):
    key = t_steps
    if key not in _CACHE:
        nc = build_core_program(t_steps)
        _CACHE[key] = BassRunner(nc, NCORES)
    return _CACHE[key]


def kernel(**inputs):
    runner = _get_runner(T)
    in_maps = [prep_core_inputs(inputs, c, T) for c in range(NCORES)]
    dev = runner.put_inputs(in_maps)
    outs = runner.run(dev)
    outT = np.asarray(outs[0])  # [8*64, 32] concat of per-core [64, 32]
    full = np.zeros((B, STATE_DIM), np.float32)
    for c in range(NCORES):
        full[c * BLOC:(c + 1) * BLOC] = outT[c * 64:(c + 1) * 64].T
    return full


# revision 14
# speedup vs baseline: 16.0186x; 16.0186x over previous
"""Trainium2 Bass kernel for nn_Dynamics: 3-layer LSTM (H=512, B=256, T=128)
+ final linear, data-parallel over batch across 8 NeuronCores.

Per-core design (B_loc=32):
  - All matmuls fp16 (fp32 PSUM accumulate), elementwise fp32, c-state fp32.
  - Gates computed transposed: gatesT [2048, 32] as 16 PSUM chunks [128, 32];
    weight strips [128,128] stationary (fp16 FWL), hT [128,32] moving.
  - Gate order reordered host-side to [i, f, o, g] so sigmoid covers one
    contiguous [128, 384] region and tanh one [128, 128].
  - Input contributions xg precomputed blockwise (S steps) into SBUF via
    big-N matmuls from the previous layer's h-sequence blocks; bias folded
    into the PSUM->SBUF copy on the scalar engine (per-partition bias AP).
  - 3 layers software-pipelined in a skewed wavefront: at wave w the core
    runs recurrence blocks (l0, w), (l1, w-1), (l2, w-2) interleaved per
    step, hiding each layer's activation/elementwise chain under the other
    layers' matmuls. Fully static unroll (no loop barriers).
  - Host-side numpy does all layout prep: x transpose to [features, (t,b)],
    weight transpose/reorder/chunking, bias sums, fp16 casts, padding.

Self-contained: only needs numpy + the installed concourse/jax environment.
"""

import numpy as np

STATE_DIM, ACTION_DIM, HIDDEN, NUM_LAYERS = 64, 32, 512, 3
B, T = 256, 128
NCORES = 8
BLOC = B // NCORES          # 32 batch rows per core
S = 8                       # steps per block
NBLK = T // S
H4 = 4 * HIDDEN
NK = HIDDEN // 128          # 4 k-chunks of the hidden dim
NM = H4 // 128              # 16 m-chunks of the gate dim

# reorder 4H rows from (i,f,g,o) to (i,f,o,g)
GATE_PERM = np.r_[0:1024, 1536:2048, 1024:1536]

# --- optimization flags (final config: fp16 matmuls, copies split ACT/DVE) ---
OPT_GP_ELEM = False    # prods/cadd elementwise on GpSimd instead of DVE
OPT_COPY_DVE = True    # alternate precompute PSUM->SBUF copies ACT/DVE
OPT_BF16 = False       # bf16 matmul operands instead of fp16
OPT_PRE_SPREAD = False  # interleave precompute m-chunks between steps

# ---------------------------------------------------------------------------
# Tile/walrus compatibility patches
# ---------------------------------------------------------------------------


def _apply_tile_patches():
    import concourse.mybir as mybir
    import concourse.tile as tile
    from concourse.vector_clock import ScopedClock

    n_spill = 24

    def _patched_dab(self, tick_clock, wait_clock):
        nc = self.nc
        nops = [nc.sync.nop(hint=f"drain_spill{i}") for i in range(n_spill)]
        drain_inst = nc.sync.drain()
        wait_clock.add_sem_waits(
            drain_inst.ins, ScopedClock({None: tick_clock.global_clock})
        )
        si = drain_inst.ins.sync_info
        waits = list(si.on_wait) if si and si.on_wait else []
        ups = list(si.on_update) if si and si.on_update else []
        if len(waits) > 1:
            spill, keep = waits[:-1], waits[-1:]
            assert len(spill) <= n_spill
            for nop, w in zip(nops, spill):
                nsi = nop.ins.sync_info
                nups = list(nsi.on_update) if nsi and nsi.on_update else []
                nop.ins.sync_info = mybir.SyncInfo(on_wait=[w], on_update=nups)
            drain_inst.ins.sync_info = mybir.SyncInfo(on_wait=keep, on_update=ups)
        nc.all_engine_barrier()
        popped = nc._tile_sem_poison_stack.pop()
        assert popped is self._sem_poison
        nc.clear_and_free_semaphores(list(self.sems.allocated().values()))
        nc.all_engine_barrier()

    tile.TileContext._drain_and_barrier = _patched_dab


_SPILL_N = [0]


def _spill_excess_waits(nc, limit=1):
    """This walrus build accepts only `limit` sync-waits per instruction;
    move excess onto injected same-engine NoOps placed just before."""
    import concourse.mybir as mybir

    for f in nc.m.functions:
        for bb in f.blocks:
            out = []
            changed = False
            for inst in bb.instructions:
                si = inst.sync_info
                waits = list(si.on_wait) if si and si.on_wait else []
                if len(waits) > limit and inst.engine != mybir.EngineType.Unassigned:
                    for w in waits[:-limit]:
                        _SPILL_N[0] += 1
                        nop = mybir.InstNoOp(
                            name=f"wait-spill-{_SPILL_N[0]}", ins=[], outs=[]
                        )
                        nop.engine = inst.engine
                        nop.sync_info = mybir.SyncInfo(on_wait=[w], on_update=[])
                        nc.register_instruction(nop)
                        out.append(nop)
                    inst.sync_info = mybir.SyncInfo(
                        on_wait=waits[-limit:],
                        on_update=list(si.on_update) if si.on_update else [],
                    )
                    changed = True
                out.append(inst)
            if changed:
                bb.instructions = out


# ---------------------------------------------------------------------------
# Bass program
# ---------------------------------------------------------------------------


def build_core_program(t_steps=T, s_blk=S, reps=1):
    import concourse.bass as bass
    import concourse.mybir as mybir
    import concourse.tile as tile

    _apply_tile_patches()

    F16 = mybir.dt.bfloat16 if OPT_BF16 else mybir.dt.float16
    F32 = mybir.dt.float32
    AF = mybir.ActivationFunctionType
    OP = mybir.AluOpType

    nblk = t_steps // s_blk
    ntok_blk = s_blk * BLOC

    nc = bass.Bass("TRN2", target_bir_lowering=False, debug=False,
                   num_devices=NCORES)

    xT = nc.declare_dram_parameter("xT", [128, t_steps * BLOC], F16,
                                   isOutput=False)
    whs, wis, bsums = [], [], []
    for l in range(NUM_LAYERS):
        whs.append(nc.declare_dram_parameter(
            f"wh{l}", [128, NK, H4], F16, isOutput=False))
        nki = 1 if l == 0 else NK
        wis.append(nc.declare_dram_parameter(
            f"wi{l}", [128, nki, H4], F16, isOutput=False))
        bsums.append(nc.declare_dram_parameter(
            f"bsum{l}", [128, NM], F32, isOutput=False))
    lin_w_in = nc.declare_dram_parameter("lin_wT", [128, NK, STATE_DIM], F16,
                                         isOutput=False)
    lin_b_in = nc.declare_dram_parameter("lin_b", [STATE_DIM, 1], F32,
                                         isOutput=False)
    outT = nc.declare_dram_parameter("outT", [STATE_DIM, BLOC], F32,
                                     isOutput=True)

    with tile.TileContext(nc) as tc:
        import contextlib
        with contextlib.ExitStack() as ctx:
            wp = ctx.enter_context(tc.tile_pool(name="weights", bufs=1))
            sp = ctx.enter_context(tc.tile_pool(name="state", bufs=1))
            pgates = ctx.enter_context(
                tc.tile_pool(name="pgates", bufs=4, space="PSUM"))
            ppre = ctx.enter_context(
                tc.tile_pool(name="ppre", bufs=2, space="PSUM"))
            plin = ctx.enter_context(
                tc.tile_pool(name="plin", bufs=1, space="PSUM"))
            xgp = [ctx.enter_context(tc.tile_pool(name=f"xg{l}", bufs=2))
                   for l in range(NUM_LAYERS)]
            hbp = [ctx.enter_context(tc.tile_pool(name=f"hb{l}", bufs=3))
                   for l in range(NUM_LAYERS)]
            tp = ctx.enter_context(tc.tile_pool(name="tmp", bufs=3))

            # --- load weights / constants ---
            wh_t, wi_t, bs_t = [], [], []
            for l in range(NUM_LAYERS):
                w = wp.tile([128, NK, H4], F16, tag=f"wh{l}")
                nc.gpsimd.dma_start(out=w, in_=whs[l][:, :, :])
                wh_t.append(w)
                nki = 1 if l == 0 else NK
                wi = wp.tile([128, nki, H4], F16, tag=f"wi{l}")
                nc.gpsimd.dma_start(out=wi, in_=wis[l][:, :, :])
                wi_t.append(wi)
                bs = wp.tile([128, NM], F32, tag=f"bs{l}")
                nc.sync.dma_start(out=bs, in_=bsums[l][:, :])
                bs_t.append(bs)
            xT_t = wp.tile([128, t_steps * BLOC], F16, tag="xT")
            nc.gpsimd.dma_start(out=xT_t, in_=xT[:, :])
            lw_t = wp.tile([128, NK, STATE_DIM], F16, tag="lw")
            nc.sync.dma_start(out=lw_t, in_=lin_w_in[:, :, :])
            lb_t = wp.tile([STATE_DIM, 1], F32, tag="lb")
            nc.sync.dma_start(out=lb_t, in_=lin_b_in[:, :])

            # --- persistent state ---
            Xs = []   # [tanh_g | c] fp32
            for l in range(NUM_LAYERS):
                X = sp.tile([128, 256], F32, tag=f"X{l}")
                nc.vector.memset(X, 0.0)
                Xs.append(X)

            # per-layer rolling H blocks and xg blocks (python-side ring)
            h_blocks = [[None] * nblk for _ in range(NUM_LAYERS)]
            xg_blocks = [[None] * nblk for _ in range(NUM_LAYERS)]

            def precompute_alloc(l, j):
                xg = xgp[l].tile([128, s_blk, H4 // NK], F16, tag=f"xg{l}")
                xg_blocks[l][j] = xg
                return xg

            def precompute_m(l, j, m, xg):
                """one m-chunk of xg block j for layer l."""
                nki = 1 if l == 0 else NK
                ps = ppre.tile([128, ntok_blk], F32, tag="pre")
                for k in range(nki):
                    if l == 0:
                        rhs = xT_t[:, j * ntok_blk:(j + 1) * ntok_blk]
                    else:
                        rhs = h_blocks[l - 1][j][:, k, :, :].rearrange(
                            "p t b -> p (t b)")
                    nc.tensor.matmul(
                        ps[:, :],
                        wi_t[l][:, k, 128 * m:128 * m + 128],
                        rhs,
                        start=(k == 0), stop=(k == nki - 1),
                    )
                if OPT_COPY_DVE and m % 2 == 1:
                    nc.vector.tensor_scalar_add(
                        out=xg[:, :, 32 * m:32 * m + 32],
                        in0=ps.rearrange("p (t b) -> p t b", t=s_blk),
                        scalar1=bs_t[l][:, m:m + 1],
                    )
                else:
                    nc.scalar.activation(
                        out=xg[:, :, 32 * m:32 * m + 32],
                        in_=ps.rearrange("p (t b) -> p t b", t=s_blk),
                        func=AF.Identity,
                        bias=bs_t[l][:, m:m + 1],
                    )

            def precompute(l, j):
                """xg block j for layer l -> SBUF fp16 [128, s_blk, 512]."""
                xg = precompute_alloc(l, j)
                for m in range(NM):
                    precompute_m(l, j, m, xg)

            def step(l, j, tau):
                """Recurrence step tau of block j, layer l."""
                X = Xs[l]
                first = (j == 0 and tau == 0)
                hb = h_blocks[l][j]
                if tau == 0:
                    hb = hbp[l].tile([128, NK, s_blk, BLOC], F16, tag=f"hb{l}")
                    h_blocks[l][j] = hb
                xg_step = xg_blocks[l][j][:, tau, :]
                if not first:
                    if tau == 0:
                        h_prev = h_blocks[l][j - 1][:, :, s_blk - 1, :]
                    else:
                        h_prev = h_blocks[l][j][:, :, tau - 1, :]
                    ps = pgates.tile([128, H4 // NK], F32, tag="gates")
                    for m in range(NM):
                        for k in range(NK):
                            nc.tensor.matmul(
                                ps[:, 32 * m:32 * m + 32],
                                wh_t[l][:, k, 128 * m:128 * m + 128],
                                h_prev[:, k, :],
                                start=(k == 0), stop=(k == NK - 1),
                            )
                    gsum = tp.tile([128, 512], F32, tag="gsum")
                    nc.vector.tensor_tensor(
                        out=gsum, in0=ps, in1=xg_step, op=OP.add)
                    gsrc = gsum
                else:
                    gsrc = xg_step
                sig = tp.tile([128, 384], F32, tag="sig")
                nc.scalar.activation(out=sig, in_=gsrc[:, 0:384],
                                     func=AF.Sigmoid)
                nc.scalar.activation(out=X[:, 0:128], in_=gsrc[:, 384:512],
                                     func=AF.Tanh)
                prods = tp.tile([128, 256], F32, tag="prods")
                elem = nc.gpsimd if OPT_GP_ELEM else nc.vector
                elem.tensor_tensor(out=prods, in0=X, in1=sig[:, 0:256],
                                   op=OP.mult)
                elem.tensor_tensor(out=X[:, 128:256], in0=prods[:, 0:128],
                                   in1=prods[:, 128:256], op=OP.add)
                tcn = tp.tile([128, 128], F32, tag="tcn")
                nc.scalar.activation(out=tcn, in_=X[:, 128:256], func=AF.Tanh)
                nc.vector.tensor_tensor(
                    out=hb[:, :, tau, :],
                    in0=sig[:, 256:384].rearrange("p (k b) -> p k b", k=NK),
                    in1=tcn.rearrange("p (k b) -> p k b", k=NK),
                    op=OP.mult)

            # --- skewed wavefront over blocks ---
            for _rep in range(reps):
                precompute(0, 0)
                for w in range(nblk + NUM_LAYERS - 1):
                    p0_tile = None
                    if OPT_PRE_SPREAD and w + 1 < nblk:
                        p0_tile = precompute_alloc(0, w + 1)
                    for tau in range(s_blk):
                        for l in range(NUM_LAYERS):
                            j = w - l
                            if 0 <= j < nblk:
                                step(l, j, tau)
                        if p0_tile is not None:
                            for m in range(2 * tau, 2 * tau + 2):
                                precompute_m(0, w + 1, m, p0_tile)
                    # remaining precompute at wave end (P1/P2 depend on the
                    # h blocks produced by this wave's steps)
                    if w + 1 < nblk and p0_tile is None:
                        precompute(0, w + 1)
                    for l in range(1, NUM_LAYERS):
                        j = w - l + 1
                        if 0 <= j < nblk:
                            precompute(l, j)

            # --- final linear: outT = lin_w @ h_last + lin_b ---
            h_last = h_blocks[NUM_LAYERS - 1][nblk - 1][:, :, s_blk - 1, :]
            pl = plin.tile([STATE_DIM, BLOC], F32, tag="lin")
            for k in range(NK):
                nc.tensor.matmul(
                    pl[:, :],
                    lw_t[:, k, :],
                    h_last[:, k, :],
                    start=(k == 0), stop=(k == NK - 1),
                )
            osb = tp.tile([STATE_DIM, BLOC], F32, tag="osb")
            nc.scalar.activation(out=osb, in_=pl, func=AF.Identity,
                                 bias=lb_t[:, 0:1])
            nc.sync.dma_start(out=outT[:, :], in_=osb)

    _spill_excess_waits(nc, limit=1)
    return nc


# ---------------------------------------------------------------------------
# Host-side input prep
# ---------------------------------------------------------------------------


def prep_core_inputs(inputs, core, t_steps=T):
    if OPT_BF16:
        import ml_dtypes
        f16 = ml_dtypes.bfloat16
    else:
        f16 = np.float16
    state = inputs["state"][core * BLOC:(core + 1) * BLOC, :t_steps]
    action = inputs["action"][core * BLOC:(core + 1) * BLOC, :t_steps]
    x = np.concatenate([state, action], axis=-1)  # [BLOC, t, 96]
    in_dim = x.shape[-1]
    xT = np.zeros((128, t_steps * BLOC), f16)
    xT[:in_dim] = x.transpose(2, 1, 0).reshape(in_dim, t_steps * BLOC)

    m = {"xT": xT}
    for l in range(NUM_LAYERS):
        w_ih = inputs[f"w_ih_{l}"][GATE_PERM]      # [2048, d_in]
        w_hh = inputs[f"w_hh_{l}"][GATE_PERM]      # [2048, 512]
        bsum = (inputs[f"b_ih_{l}"] + inputs[f"b_hh_{l}"])[GATE_PERM]
        wt = w_hh.T.reshape(NK, 128, H4).transpose(1, 0, 2)
        m[f"wh{l}"] = wt.astype(f16)
        if l == 0:
            wi = np.zeros((128, 1, H4), np.float32)
            wi[:w_ih.shape[1], 0] = w_ih.T
        else:
            wi = w_ih.T.reshape(NK, 128, H4).transpose(1, 0, 2)
        m[f"wi{l}"] = wi.astype(f16)
        m[f"bsum{l}"] = np.ascontiguousarray(
            bsum.reshape(NM, 128).T).astype(np.float32)
    m["lin_wT"] = np.ascontiguousarray(
        inputs["lin_w"].T.reshape(NK, 128, STATE_DIM).transpose(1, 0, 2)
    ).astype(f16)
    m["lin_b"] = inputs["lin_b"].reshape(STATE_DIM, 1).astype(np.float32)
    return m


# ---------------------------------------------------------------------------
# PJRT runner (axon path), adapted from concourse.bass2jax.run_bass_via_pjrt
# ---------------------------------------------------------------------------


class BassRunner:
    def __init__(self, nc, n_cores):
        import jax
        from jax.sharding import Mesh, PartitionSpec
        from jax.experimental.shard_map import shard_map
        import concourse.mybir as mybir
        from concourse.bass2jax import (
            _bass_exec_p, install_neuronx_cc_hook, partition_id_tensor)

        install_neuronx_cc_hook()
        self.jax = jax
        self.nc = nc
        self.n_cores = n_cores
        partition_name = (
            nc.partition_id_tensor.name if nc.partition_id_tensor else None)

        in_names, out_names, out_avals, zero_outs = [], [], [], []
        for alloc in nc.m.functions[0].allocations:
            if not isinstance(alloc, mybir.MemoryLocationSet):
                continue
            name = alloc.memorylocations[0].name
            if alloc.kind == "ExternalInput":
                if name != partition_name:
                    in_names.append(name)
            elif alloc.kind == "ExternalOutput":
                out_names.append(name)
                shape = tuple(alloc.tensor_shape)
                dtype = mybir.dt.np(alloc.dtype)
                out_avals.append(jax.core.ShapedArray(shape, dtype))
                zero_outs.append(np.zeros(shape, dtype))
        self.in_names = in_names
        self.out_names = out_names
        self.zero_outs = zero_outs
        n_params, n_outs = len(in_names), len(out_avals)

        all_in_names = list(in_names) + list(out_names)
        if partition_name is not None:
            all_in_names.append(partition_name)

        def _body(*args):
            operands = list(args)
            if partition_name is not None:
                operands.append(partition_id_tensor())
            return tuple(_bass_exec_p.bind(
                *operands,
                out_avals=tuple(out_avals),
                in_names=tuple(all_in_names),
                out_names=tuple(out_names),
                lowering_input_output_aliases=(),
                sim_require_finite=True,
                sim_require_nnan=True,
                nc=nc,
            ))

        devices = jax.devices()[:n_cores]
        mesh = Mesh(np.asarray(devices), ("core",))
        self.fn = jax.jit(
            shard_map(_body, mesh=mesh,
                      in_specs=(PartitionSpec("core"),) * (n_params + n_outs),
                      out_specs=(PartitionSpec("core"),) * n_outs,
                      check_rep=False),
            donate_argnums=tuple(range(n_params, n_params + n_outs)),
            keep_unused=True,
        )

    def put_inputs(self, in_maps):
        return [
            self.jax.device_put(np.concatenate(
                [np.asarray(m[name]) for m in in_maps], axis=0))
            for name in self.in_names
        ]

    def _zeros(self):
        jnp = self.jax.numpy
        return [jnp.zeros((self.n_cores * z.shape[0], *z.shape[1:]), z.dtype)
                for z in self.zero_outs]

    def run(self, dev_inputs):
        outs = self.fn(*dev_inputs, *self._zeros())
        self.jax.block_until_ready(outs)
        return outs

    def time_run(self, dev_inputs, iters=10, warmup=2):
        import time as _time
        for _ in range(warmup):
            self.run(dev_inputs)
        ts = []
        for _ in range(iters):
            zs = self._zeros()
            self.jax.block_until_ready(zs)
            t0 = _time.perf_counter()
            outs = self.fn(*dev_inputs, *zs)
            self.jax.block_until_ready(outs)
            ts.append(_time.perf_counter() - t0)
        return min(ts), ts


# ---------------------------------------------------------------------------
# Public entry point
# ---------------------------------------------------------------------------

_CACHE = {}


def _get_runner(t_steps=T):
    key = t_steps
    if key not in _CACHE:
        nc = build_core_program(t_steps)
        _CACHE[key] = BassRunner(nc, NCORES)
    return _CACHE[key]


def kernel(**inputs):
    inputs = {k: np.asarray(v) for k, v in inputs.items()}
    runner = _get_runner(T)
    in_maps = [prep_core_inputs(inputs, c, T) for c in range(NCORES)]
    dev = runner.put_inputs(in_maps)
    outs = runner.run(dev)
    outT = np.asarray(outs[0])  # [8*64, 32] concat of per-core [64, 32]
    full = np.zeros((B, STATE_DIM), np.float32)
    for c in range(NCORES):
        full[c * BLOC:(c + 1) * BLOC] = outT[c * 64:(c + 1) * 64].T
    return full
